# revision 18
# baseline (speedup 1.0000x reference)
"""Trainium2 Bass kernel for nn_ConnectivityGraphGenerator.

Data-parallel over batch B=128: 16 graphs per core on 8 NeuronCores.

Math restructure (vs the reference's gather/scatter formulation):
  - The edge index is the FIXED complete upper-triangular graph on N=64
    nodes, so the PyG mean aggregation is a prefix-mean over nodes:
    agg[j] = mean_{i<j} x[i]. Computed as one matmul with a constant
    [N,N] strictly-upper-triangular matrix whose column j is 1/max(j,1).
  - The edge MLP heads decompose: ef@W = h[src]@W_top + h[dst]@W_bot,
    so we compute per-node projections A=h@W_top, B=h@W_bot (+bias into
    B) and form per-edge values as a broadcast outer sum A[:,i]+B[:,j]
    via stride-0 access patterns — no gathers.
  - Everything is kept feature-major (d on partitions, (i,j) on the free
    axis) so the d-reduction is a ones-matvec on the PE.
  - The (i,j) grid is processed in 4 upper-triangular j-blocks of 16
    columns (i < 16*(k+1)), cutting dense-pair work 4096 -> 2560 and
    giving the Tile scheduler fine-grained blocks to pipeline across
    DVE/ACT/GPSIMD/PE.
  - gumbel-softmax: exp(2g) = 1/ln(u)^2 exactly, so per-edge we emit
    ez = exp(2*sigmoid(w))/ln(u)^2 and v = sim*ez; the host divides by
    the global sum of ez over real edges (softmax over the full B*E
    vector couples all cores; the division is part of unsharding).
  - softplus has no ACT table in this build: V = Ln(Exp(P) + 1) (exact;
    |P| < 3 so exp cannot overflow). The +1e-6 variance epsilon is
    dropped: min softplus here is ~0.075, so eps shifts the result by
    <2e-5 relative, far below fp32 noise in the final output.
"""

import numpy as np

import concourse.bacc as bacc
import concourse.bass as bass
import concourse.mybir as mybir
import concourse.tile as tile
from concourse.bass_utils import run_bass_kernel_spmd
from concourse.tile_rust import add_dep_helper

F32 = mybir.dt.float32
AF = mybir.ActivationFunctionType
ALU = mybir.AluOpType

B, N, T = 128, 64, 256
IN, H, OUT = N + T, 256, 128
E = N * (N - 1) // 2  # 2016
NCORES = 8
G = B // NCORES  # 16 graphs per core

# Upper-triangular j-blocks: block k covers j in [16k, 16k+16), i in [0, 16k+16)
JW = 16
NBLK = N // JW
BLOCKS = []  # (j0, iw, off, F)
_off = 0
for _k in range(NBLK):
    _iw = JW * (_k + 1)
    BLOCKS.append((JW * _k, _iw, _off, _iw * JW))
    _off += _iw * JW
NB = _off  # 2560 blocked pair slots per graph


def _patch_act_tables():
    """Order activation tables so Exp and Ln resolve to the combined
    natural_log_exp_and_others set (the default greedy pick alternates
    exp_and_others / natural_log, reloading tables around every big op)."""
    import concourse.hw_specs as hw_specs

    if getattr(hw_specs.get_activation_tables, "_patched", False):
        return
    _orig = hw_specs.get_activation_tables

    def patched(arch):
        t = _orig(arch)
        pref = ["natural_log_exp_and_others", "sigmoid_and_others"]
        out = {k: t[k] for k in pref if k in t}
        out.update({k: v for k, v in t.items() if k not in out})
        return out

    patched._patched = True
    hw_specs.get_activation_tables = patched
    bacc.get_activation_tables = patched


def _body(ctx, tc):
    nc = tc.nc
    x_d = nc.dram_tensor("x", [G, N, IN], F32, kind="ExternalInput").ap()
    u_d = nc.dram_tensor("u", [G, NB], F32, kind="ExternalInput").ap()
    wg_d = nc.dram_tensor("w_gnn", [IN, H], F32, kind="ExternalInput").ap()
    bg_d = nc.dram_tensor("b_gnn", [H, 1], F32, kind="ExternalInput").ap()
    wm_d = nc.dram_tensor("w_mean", [2 * H, OUT], F32, kind="ExternalInput").ap()
    bm_d = nc.dram_tensor("b_mean", [OUT, 1], F32, kind="ExternalInput").ap()
    wv_d = nc.dram_tensor("w_var", [2 * H, OUT], F32, kind="ExternalInput").ap()
    bv_d = nc.dram_tensor("b_var", [OUT, 1], F32, kind="ExternalInput").ap()
    ww_d = nc.dram_tensor("w_w", [2 * H, 1], F32, kind="ExternalInput").ap()
    bw_d = nc.dram_tensor("b_w", [G, 1], F32, kind="ExternalInput").ap()
    v_d = nc.dram_tensor("v", [G, NB], F32, kind="ExternalOutput").ap()
    ez_d = nc.dram_tensor("ez", [G, NB], F32, kind="ExternalOutput").ap()

    singles = ctx.enter_context(tc.tile_pool(name="singles", bufs=1))

    # --- constants ---
    # lts[i, j] = 1/max(j,1) if i < j else 0  -> x.T @ lts = prefix-mean
    lts = singles.tile([N, N], F32)
    tmp = singles.tile([N, N], F32)
    nc.gpsimd.iota(
        tmp[:],
        pattern=[[1, N]],
        base=0,
        channel_multiplier=0,
        allow_small_or_imprecise_dtypes=True,
    )
    nc.vector.tensor_scalar_max(tmp[:], tmp[:], 1.0)
    nc.vector.reciprocal(tmp[:], tmp[:])
    nc.gpsimd.affine_select(
        out=lts[:],
        in_=tmp[:],
        compare_op=ALU.is_gt,
        fill=0.0,
        base=0,
        pattern=[[1, N]],  # value = j - i ; keep where > 0
        channel_multiplier=-1,
    )
    # lhsT for the d-reduction: sum_d * (-1/(2*OUT)) => -0.5*mean_d
    negones = singles.tile([OUT, 1], F32)
    nc.vector.memset(negones[:], -1.0 / (2 * OUT))

    # --- weights ---
    wg_t = singles.tile([128, 3, H], F32)
    nc.sync.dma_start(wg_t[:, 0, :], wg_d[0:128, :])
    nc.sync.dma_start(wg_t[:, 1, :], wg_d[128:256, :])
    nc.sync.dma_start(wg_t[:64, 2, :], wg_d[256:320, :])
    wm_t = singles.tile([128, 4, OUT], F32)
    wv_t = singles.tile([128, 4, OUT], F32)
    ww_t = singles.tile([128, 4, 1], F32)
    for k in range(4):
        nc.sync.dma_start(wm_t[:, k, :], wm_d[k * 128 : (k + 1) * 128, :])
        nc.sync.dma_start(wv_t[:, k, :], wv_d[k * 128 : (k + 1) * 128, :])
        nc.sync.dma_start(ww_t[:, k, :], ww_d[k * 128 : (k + 1) * 128, :])
    bg_t = singles.tile([128, 2, 1], F32)
    nc.sync.dma_start(bg_t[:, 0, :], bg_d[0:128, :])
    nc.sync.dma_start(bg_t[:, 1, :], bg_d[128:256, :])
    bm_t = singles.tile([OUT, 1], F32)
    nc.sync.dma_start(bm_t[:], bm_d[:])
    bv_t = singles.tile([OUT, 1], F32)
    nc.sync.dma_start(bv_t[:], bv_d[:])
    bw_t = singles.tile([G, 1], F32)
    nc.sync.dma_start(bw_t[:], bw_d[:])

    # --- pools ---
    xp = ctx.enter_context(tc.tile_pool(name="xp", bufs=2))
    aggp = ctx.enter_context(tc.tile_pool(name="aggp", bufs=2))
    hp = ctx.enter_context(tc.tile_pool(name="hp", bufs=2))
    headp = ctx.enter_context(tc.tile_pool(name="headp", bufs=2))
    bigp = ctx.enter_context(tc.tile_pool(name="bigp", bufs=4))
    rowp = ctx.enter_context(tc.tile_pool(name="rowp", bufs=3))
    psm = ctx.enter_context(tc.tile_pool(name="psm", bufs=5, space="PSUM"))
    psr = ctx.enter_context(tc.tile_pool(name="psr", bufs=3, space="PSUM"))
    tailp = ctx.enter_context(tc.tile_pool(name="tailp", bufs=1))

    s_all = tailp.tile([G, NB], F32)
    wa_all = tailp.tile([G, N], F32)
    wb_all = tailp.tile([G, N], F32)

    # Graph-group phasing: all Exp work for KG graphs, then all Ln work,
    # so the ACT engine switches its (per-anchor-function) table set twice
    # per group instead of twice per block. Square/Relu/Identity/Copy live
    # in every table set and never force a load.
    KG = 4
    ORDER_ACT = globals().get('_ORDER_ACT', True)

    def front(g):
        """x load, prefix-mean agg, GNN layer, head projections for graph g."""
        xt = xp.tile([N, IN], F32, tag="xt")
        nc.sync.dma_start(xt[:], x_d[g])
        agg_s = aggp.tile([128, 3, N], F32, tag="agg")
        for c in range(3):
            kp = 128 if c < 2 else 64
            ps = psm.tile([128, N], F32, tag="ps")
            nc.tensor.matmul(
                ps[:kp],
                lhsT=xt[:, c * 128 : c * 128 + kp],
                rhs=lts[:],
                start=True,
                stop=True,
            )
            nc.vector.tensor_copy(agg_s[:kp, c, :], ps[:kp])
        hT = hp.tile([128, 2, N], F32, tag="h")
        for c in range(2):
            ph = psm.tile([128, N], F32, tag="ps")
            for k in range(3):
                kp = 128 if k < 2 else 64
                nc.tensor.matmul(
                    ph[:],
                    lhsT=wg_t[:kp, k, c * 128 : (c + 1) * 128],
                    rhs=agg_s[:kp, k, :],
                    start=(k == 0),
                    stop=(k == 2),
                )
            nc.scalar.activation(hT[:, c, :], ph[:], AF.Relu, bias=bg_t[:, c, :])
        heads = {}
        for nm, w_t, koff, bias_t in (
            ("at", wm_t, 0, None),
            ("bt", wm_t, 2, bm_t),
            ("ct", wv_t, 0, None),
            ("dt", wv_t, 2, bv_t),
        ):
            dst = headp.tile([OUT, N], F32, tag=nm)
            pp = psm.tile([OUT, N], F32, tag="ps")
            for k in (0, 1):
                nc.tensor.matmul(
                    pp[:],
                    lhsT=w_t[:, koff + k, :],
                    rhs=hT[:, k, :],
                    start=(k == 0),
                    stop=(k == 1),
                )
            if bias_t is None:
                nc.scalar.activation(dst[:], pp[:], AF.Identity)
            else:
                nc.scalar.activation(dst[:], pp[:], AF.Identity, bias=bias_t[:])
            heads[nm] = dst
        # scalar edge-weight head; compute engines cannot start at arbitrary
        # partitions, so stage at partition 0 and DMA into the per-graph row.
        w_row = rowp.tile([1, 2 * N], F32, tag="wrow")
        for koff, dst_rows, col in ((0, wa_all, 0), (2, wb_all, N)):
            pw = psm.tile([1, N], F32, tag="ps")
            for k in (0, 1):
                nc.tensor.matmul(
                    pw[:],
                    lhsT=ww_t[:, koff + k, :],
                    rhs=hT[:, k, :],
                    start=(k == 0),
                    stop=(k == 1),
                )
            nc.vector.tensor_copy(w_row[:, col : col + N], pw[:])
            nc.sync.dma_start(dst_rows[g : g + 1, :], w_row[:, col : col + N])
        return heads

    prev_last_ln = [None]  # ACT-order anchor across groups
    for g0 in range(0, G, KG):
        group = list(range(g0, min(g0 + KG, G)))
        tiles = {}
        last_exp = [None]
        # phase A: fronts, M/P broadcast adds, exp(P), M^2
        for g in group:
            heads = front(g)
            at, bt, ct, dt = heads["at"], heads["bt"], heads["ct"], heads["dt"]
            for bi, (j0, iw, off, F) in enumerate(BLOCKS):
                m_t = bigp.tile([OUT, F], F32, tag=f"m{bi}")
                p_t = bigp.tile([OUT, F], F32, tag=f"p{bi}")
                m3 = m_t[:].rearrange("p (a b) -> p a b", a=iw)
                p3 = p_t[:].rearrange("p (a b) -> p a b", a=iw)
                # P on DVE so Exp (table-ordered) is fed promptly; M on
                # GPSIMD since Square runs from any table set.
                nc.gpsimd.tensor_add(
                    m3,
                    at[:, :iw, None].broadcast_to([OUT, iw, JW]),
                    bt[:, None, j0 : j0 + JW].broadcast_to([OUT, iw, JW]),
                )
                nc.vector.tensor_add(
                    p3,
                    ct[:, :iw, None].broadcast_to([OUT, iw, JW]),
                    dt[:, None, j0 : j0 + JW].broadcast_to([OUT, iw, JW]),
                )
                ei = nc.scalar.activation(p_t[:], p_t[:], AF.Exp)
                if ORDER_ACT and prev_last_ln[0] is not None:
                    # pin ACT dispatch order so Exp/Ln phases don't
                    # interleave across groups (each flip reloads the
                    # ~2.7us activation table set)
                    add_dep_helper(ei.ins, prev_last_ln[0], sync=False,
                                   reason="act table phase order")
                last_exp[0] = ei.ins
                nc.gpsimd.tensor_mul(m_t[:], m_t[:], m_t[:])
                tiles[(g, bi)] = (m_t, p_t)
        # phase B: ln -> 1/softplus, Q = M^2*Vr, PE reduce, evacuate
        for g in group:
            s_row = rowp.tile([1, NB], F32, tag="srow")
            nev = 0
            for bi, (j0, iw, off, F) in enumerate(BLOCKS):
                m_t, p_t = tiles.pop((g, bi))
                li = nc.scalar.activation(p_t[:], p_t[:], AF.Ln, bias=1.0)
                if ORDER_ACT:
                    add_dep_helper(li.ins, last_exp[0], sync=False,
                                   reason="act table phase order")
                prev_last_ln[0] = li.ins
                nc.vector.reciprocal_approx_fast(p_t[:], p_t[:])
                nc.vector.tensor_mul(m_t[:], m_t[:], p_t[:])
                for c0 in range(0, F, 512):
                    cw = min(512, F - c0)
                    sr = psr.tile([1, 512], F32, tag="sr")
                    nc.tensor.matmul(
                        sr[:, :cw],
                        lhsT=negones[:],
                        rhs=m_t[:, c0 : c0 + cw],
                        start=True,
                        stop=True,
                    )
                    dst = s_row[:, off + c0 : off + c0 + cw]
                    if nev % 3 == 2:
                        nc.vector.tensor_copy(dst, sr[:, :cw])
                    else:
                        nc.scalar.activation(dst, sr[:, :cw], AF.Copy)
                    nev += 1
            nc.sync.dma_start(s_all[g : g + 1, :], s_row[:])

    # --- tail: all 16 graphs at once, [G, NB] blocked layout ---
    u_t = tailp.tile([G, NB], F32)
    nc.sync.dma_start(u_t[:], u_d[:])
    wden = tailp.tile([G, NB], F32)
    for j0, iw, off, F in BLOCKS:
        w3 = wden[:, off : off + F].rearrange("g (a b) -> g a b", a=iw)
        nc.vector.scalar_tensor_tensor(
            out=w3,
            in0=wa_all[:, :iw, None].broadcast_to([G, iw, JW]),
            scalar=bw_t[:],
            in1=wb_all[:, None, j0 : j0 + JW].broadcast_to([G, iw, JW]),
            op0=ALU.add,
            op1=ALU.add,
        )
    ez_t = tailp.tile([G, NB], F32)
    HB = NB // 2
    for h0 in (0, HB):  # column halves so ACT/DVE overlap within the tail
        sl = slice(h0, h0 + HB)
        nc.scalar.activation(wden[:, sl], wden[:, sl], AF.Sigmoid)
        nc.scalar.activation(u_t[:, sl], u_t[:, sl], AF.Ln)
        nc.vector.tensor_mul(u_t[:, sl], u_t[:, sl], u_t[:, sl])  # ln(u)^2
        nc.vector.reciprocal_approx_fast(u_t[:, sl], u_t[:, sl])
        nc.scalar.activation(wden[:, sl], wden[:, sl], AF.Exp, scale=2.0)
        nc.scalar.activation(s_all[:, sl], s_all[:, sl], AF.Exp)  # sim
        nc.vector.tensor_mul(ez_t[:, sl], wden[:, sl], u_t[:, sl])  # exp(z)
        nc.sync.dma_start(ez_d[:, sl], ez_t[:, sl])
        nc.vector.tensor_mul(wden[:, sl], ez_t[:, sl], s_all[:, sl])
        nc.sync.dma_start(v_d[:, sl], wden[:, sl])


_NC_CACHE = None


def _build_nc():
    global _NC_CACHE
    if _NC_CACHE is not None:
        return _NC_CACHE
    from contextlib import ExitStack

    if not globals().get('_NO_TABLE_PATCH'):
        _patch_act_tables()
    nc = bacc.Bacc(
        "TRN2",
        target_bir_lowering=False,
        debug=False,
        enable_asserts=False,
        num_devices=NCORES,
    )
    with tile.TileContext(nc) as tc, ExitStack() as ctx:
        _body(ctx, tc)
    nc.compile()
    _NC_CACHE = nc
    return nc


def _edge_positions():
    """Blocked-layout position of each upper-tri edge (i,j)."""
    iu0, iu1 = np.triu_indices(N, k=1)
    offs = np.array([b[2] for b in BLOCKS])
    pos = offs[iu1 // JW] + iu0 * JW + (iu1 % JW)
    return iu0, iu1, pos


def _make_in_maps(
    x_topology, x_temporal, gumbel_u, W_gnn, b_gnn, W_mean, b_mean, W_var, b_var, W_w, b_w
):
    f = np.float32
    x_full = np.concatenate(
        [np.asarray(x_topology, f), np.asarray(x_temporal, f)], axis=-1
    )  # [B, N, IN]
    _, _, pos = _edge_positions()
    u_blk = np.full((B, NB), 0.5, f)
    u_blk[:, pos] = np.asarray(gumbel_u, f).reshape(B, E)
    shared = {
        "w_gnn": np.ascontiguousarray(W_gnn, f),
        "b_gnn": np.asarray(b_gnn, f).reshape(H, 1),
        "w_mean": np.ascontiguousarray(W_mean, f),
        "b_mean": np.asarray(b_mean, f).reshape(OUT, 1),
        "w_var": np.ascontiguousarray(W_var, f),
        "b_var": np.asarray(b_var, f).reshape(OUT, 1),
        "w_w": np.ascontiguousarray(W_w, f),
        "b_w": np.full((G, 1), np.asarray(b_w, f).reshape(-1)[0], f),
    }
    in_maps = []
    for c in range(NCORES):
        sl = slice(c * G, (c + 1) * G)
        m = dict(shared)
        m["x"] = np.ascontiguousarray(x_full[sl])
        m["u"] = np.ascontiguousarray(u_blk[sl])
        in_maps.append(m)
    return in_maps


def _run_raw(in_maps, trace=False, **kw):
    nc = _build_nc()
    return run_bass_kernel_spmd(
        nc, in_maps, core_ids=list(range(NCORES)), trace=trace, **kw
    )


def kernel(**inputs) -> np.ndarray:
    in_maps = _make_in_maps(**inputs)
    res = _run_raw(in_maps)
    iu0, iu1, pos = _edge_positions()
    v = np.concatenate([r["v"] for r in res.results], axis=0)  # [B, NB]
    ez = np.concatenate([r["ez"] for r in res.results], axis=0)
    vals_v = v[:, pos]
    gsum = ez[:, pos].sum(dtype=np.float32)
    adj = np.zeros((B, N, N), np.float32)
    adj[iu0 * 0 + np.arange(B)[:, None], iu0[None, :], iu1[None, :]] = vals_v / gsum
    return adj
